# revision 1
# baseline (speedup 1.0000x reference)
"""CLIP-style contrastive loss on 8 Trainium2 NeuronCores.

Math: with labels = eye(B), the Keras CCE(prob, eye) loss only depends on the
diagonal of the softmax probabilities:
    sum_ij prob_ij * logclip_ij = tr * log(1-eps) + (B - tr) * log(eps)
where tr = trace(prob). And since |logits| <= exp(temperature) (cosine
similarities), softmax needs no max subtraction; prob_ii = E_ii / rowsum(E)
with E = exp(scale * S), S = l2norm(left) @ l2norm(right).T.

Sharding: 4x2 grid over the BxB similarity matrix. Core (p, q) owns
left rows [1024p, 1024p+1024) x right rows [2048q, 2048q+2048). Per 128-row
tile: load fp32, sqsum fused into the ScalarE square pass, rsqrt via
Ln+Exp(-0.5) (stays in one ACT table set), normalize+cast to bf16 on VectorE,
then one SBUF->SBUF xbar DMA transpose per tile scatters all 8 k-slices into
the d-major operand tile. bf16 matmuls accumulate k in PSUM; exp+rowsum fuse
into one ScalarE pass per [128,1024] PSUM pair; column sums via ones-vector
matmul; diagonal via identity-mask mul+reduce. Host combines partial sums.
"""

import math
import numpy as np

import concourse.bass as bass
import concourse.mybir as mybir
import concourse.tile as tile
from concourse import bacc
from concourse.bass import ds, ts
from concourse.masks import make_identity

B = 4096
D = 1024
EPS = 1e-7
WEIGHT = 1.0

PGRID = 4  # row groups (left)
QGRID = 2  # col groups (right)
LROWS = B // PGRID   # 1024 left rows per core
RROWS = B // QGRID   # 2048 right rows per core
KT = D // 128        # 8 contraction k-tiles
MT = LROWS // 128    # 8 m row-tiles
RT = RROWS // 128    # 16 right row-tiles

AF = mybir.ActivationFunctionType
F32 = mybir.dt.float32
BF16 = mybir.dt.bfloat16


def _build_body(tc, lblk, rblk, temp, rowsum_o, colsum_o, diag_o):
    nc = tc.nc
    from contextlib import ExitStack

    with ExitStack() as ctx:
        const_pool = ctx.enter_context(tc.tile_pool(name="const", bufs=1))
        small = ctx.enter_context(tc.tile_pool(name="small", bufs=1))
        nat_pool = ctx.enter_context(tc.tile_pool(name="nat", bufs=6))
        sq_pool = ctx.enter_context(tc.tile_pool(name="sq", bufs=2))
        nrm_pool = ctx.enter_context(tc.tile_pool(name="nrm", bufs=6))
        op_pool = ctx.enter_context(tc.tile_pool(name="op", bufs=1))
        e_pool = ctx.enter_context(tc.tile_pool(name="E", bufs=MT))
        dram_pool = ctx.enter_context(tc.tile_pool(name="scr", bufs=1, space="DRAM"))
        ps_mm = ctx.enter_context(tc.tile_pool(name="psmm", bufs=3, space="PSUM"))
        ps_cs = ctx.enter_context(tc.tile_pool(name="pscs", bufs=2, space="PSUM"))

        # ---- constants ----
        eye = const_pool.tile([128, 128], BF16, tag="eye")
        make_identity(nc, eye[:])
        ones_row = const_pool.tile([1, 128], F32, tag="ones_row")
        nc.vector.memset(ones_row[:], 1.0)
        ones_col = const_pool.tile([128, 1], BF16, tag="ones_col")
        nc.vector.memset(ones_col[:], 1.0)

        # ---- escale = exp(temperature) broadcast to 128 partitions ----
        t_sb = small.tile([1, 1], F32, tag="t_sb")
        nc.sync.dma_start(t_sb[:], temp.rearrange("(a b) -> a b", a=1))
        esc11 = small.tile([1, 1], F32, tag="esc11")
        nc.scalar.activation(esc11[:], t_sb[:], AF.Exp)
        esc_ps = ps_mm.tile([128, 1], F32, tag="ps", name="escps")
        nc.tensor.matmul(esc_ps[:], ones_row[:], esc11[:], start=True, stop=True)
        escale = small.tile([128, 1], F32, tag="escale")
        nc.vector.tensor_copy(escale[:], esc_ps[:])

        # ---- accumulators / operands ----
        rowacc = small.tile([128, MT * 2], F32, tag="rowacc")
        diagacc = small.tile([128, MT * 2], F32, tag="diagacc")
        colsb = small.tile([1, RROWS], F32, tag="colsb")
        sqL = small.tile([128, MT], F32, tag="sqL")
        sqR = small.tile([128, RT], F32, tag="sqR")
        lnT = op_pool.tile([128, MT, KT, 128], BF16, tag="lnT")
        rnT = op_pool.tile([128, RT, KT, 128], BF16, tag="rnT")
        etiles = [e_pool.tile([128, RROWS], BF16, tag="E", name=f"E{m}")
                  for m in range(MT)]

        def tower_group(src, sq_all, dstT, tiles, gname, xpose_engine):
            """Load 4 row-tiles, fused square+sqsum, rsqrt, normalize, then one
            quad xbar transpose scattering all 8 k-slices of all 4 tiles."""
            g0 = tiles[0] // 2
            nats = []
            for t in tiles:
                nat = nat_pool.tile([128, D], F32, tag="nat", name=f"nat{gname}{t}")
                nc.gpsimd.dma_start(nat[:], src[ts(t, 128), :])
                sqd = sq_pool.tile([128, D], F32, tag="sq", name=f"sq{gname}{t}")
                nc.scalar.activation(sqd[:], nat[:], AF.Square,
                                     accum_out=sq_all[:, ds(t, 1)])
                nats.append(nat)
            n = len(tiles)
            mx = small.tile([128, n], F32, tag=f"mx{gname}{g0}")
            nc.vector.tensor_scalar_max(mx[:], sq_all[:, ds(tiles[0], n)], EPS)
            sr = small.tile([128, n], F32, tag=f"sr{gname}{g0}")
            nc.scalar.activation(sr[:], mx[:], AF.Sqrt)
            inv = small.tile([128, n], F32, tag=f"inv{gname}{g0}")
            nc.vector.reciprocal(inv[:], sr[:])
            nbq = nrm_pool.tile([128, n, D], BF16, tag="nrm", name=f"nbq{gname}{g0}")
            for i, t in enumerate(tiles):
                nc.vector.tensor_scalar_mul(nbq[:, i, :], nats[i][:],
                                            inv[:, ds(i, 1)])
            xpose_engine.dma_start_transpose(dstT[:, ds(tiles[0], len(tiles))], nbq[:])

        # ---- towers, ordered so the first matmuls unblock earliest ----
        tower_group(lblk, sqL, lnT, [0, 1, 2, 3], "L", nc.sync)
        tower_group(rblk, sqR, rnT, [0, 1, 2, 3], "R", nc.sync)
        tower_group(rblk, sqR, rnT, [4, 5, 6, 7], "R", nc.sync)
        tower_group(lblk, sqL, lnT, [4, 5, 6, 7], "L", nc.sync)
        tower_group(rblk, sqR, rnT, [8, 9, 10, 11], "R", nc.sync)
        tower_group(rblk, sqR, rnT, [12, 13, 14, 15], "R", nc.sync)

        # ---- matmul + fused exp/rowsum, in column halves h ----
        for h in range(2):
            for mt in range(MT):
                ps = ps_mm.tile([128, 1024], F32, tag="ps", name=f"ps{h}_{mt}")
                for j in range(2):
                    for k in range(KT):
                        nc.tensor.matmul(
                            ps[:, ds(j * 512, 512)],
                            lnT[:, mt, k, :],
                            rnT[:, ds(h * 8 + j * 4, 4), k, :],
                            start=(k == 0), stop=(k == KT - 1),
                        )
                nc.scalar.activation(
                    etiles[mt][:, ds(h * 1024, 1024)], ps[:], AF.Exp,
                    scale=escale[:, 0:1],
                    accum_out=rowacc[:, ds(mt * 2 + h, 1)],
                )

            # column sums of exp over this half (partition reduce via matmul)
            for j in range(2):
                cps = ps_cs.tile([1, 512], F32, tag="cs", name=f"cps{h}_{j}")
                for mt in range(MT):
                    nc.tensor.matmul(
                        cps[:], ones_col[:],
                        etiles[mt][:, ds(h * 1024 + j * 512, 512)],
                        start=(mt == 0), stop=(mt == MT - 1),
                    )
                nc.vector.tensor_copy(colsb[:, ds(h * 1024 + j * 512, 512)], cps[:])

            # diagonal candidates at column offset 1024*h
            for mt in range(MT):
                dscr = sq_pool.tile([128, 128], F32, tag="dscr",
                                    name=f"dscr{h}_{mt}")
                nc.vector.tensor_mul(
                    dscr[:], etiles[mt][:, ds(h * 1024 + mt * 128, 128)], eye[:])
                nc.vector.tensor_reduce(
                    diagacc[:, ds(mt * 2 + h, 1)], dscr[:],
                    axis=mybir.AxisListType.X, op=mybir.AluOpType.add)

        # ---- finalize outputs ----
        rs = small.tile([128, MT], F32, tag="rs")
        nc.vector.tensor_reduce(
            rs[:], rowacc[:].rearrange("p (m c) -> p m c", c=2),
            axis=mybir.AxisListType.X, op=mybir.AluOpType.add,
        )
        nc.sync.dma_start(rowsum_o[:], rs[:])
        nc.sync.dma_start(colsum_o.rearrange("(a c) -> a c", a=1), colsb[:])
        nc.sync.dma_start(diag_o[:], diagacc[:])


_CACHED = {}


def _get_program():
    if "nc" in _CACHED:
        return _CACHED["nc"]
    nc = bacc.Bacc("TRN2", target_bir_lowering=False, debug=False,
                   num_devices=PGRID * QGRID)
    lblk = nc.dram_tensor("lblk", [LROWS, D], F32, kind="ExternalInput").ap()
    rblk = nc.dram_tensor("rblk", [RROWS, D], F32, kind="ExternalInput").ap()
    temp = nc.dram_tensor("temp", [1], F32, kind="ExternalInput").ap()
    rowsum_o = nc.dram_tensor("rowsum", [128, MT], F32, kind="ExternalOutput").ap()
    colsum_o = nc.dram_tensor("colsum", [RROWS], F32, kind="ExternalOutput").ap()
    diag_o = nc.dram_tensor("diag", [128, MT * 2], F32, kind="ExternalOutput").ap()
    with tile.TileContext(nc) as tc:
        _build_body(tc, lblk, rblk, temp, rowsum_o, colsum_o, diag_o)
    nc.compile()
    _CACHED["nc"] = nc
    return nc


def _run(inputs, trace=False):
    from concourse.bass_utils import run_bass_kernel_spmd

    nc = _get_program()
    left = np.ascontiguousarray(inputs["left"], dtype=np.float32)
    right = np.ascontiguousarray(inputs["right"], dtype=np.float32)
    temp = np.ascontiguousarray(inputs["temperature"], dtype=np.float32)

    in_maps = []
    for p in range(PGRID):
        for q in range(QGRID):
            in_maps.append({
                "lblk": left[p * LROWS:(p + 1) * LROWS],
                "rblk": right[q * RROWS:(q + 1) * RROWS],
                "temp": temp,
            })
    res = run_bass_kernel_spmd(nc, in_maps, core_ids=list(range(PGRID * QGRID)),
                               trace=trace)
    return res


def _combine(results):
    rowsum = np.zeros(B, dtype=np.float64)
    colsum = np.zeros(B, dtype=np.float64)
    diag = np.zeros(B, dtype=np.float64)
    for p in range(PGRID):
        for q in range(QGRID):
            r = results[p * QGRID + q]
            rs = r["rowsum"].astype(np.float64)  # [128, MT]
            rowsum[p * LROWS:(p + 1) * LROWS] += rs.T.reshape(-1)
            colsum[q * RROWS:(q + 1) * RROWS] += r["colsum"].astype(np.float64)
            delta = LROWS * p - RROWS * q
            if delta in (0, 1024):
                a = delta // 1024
                d = r["diag"].astype(np.float64).reshape(128, MT, 2)[:, :, a]
                diag[p * LROWS:(p + 1) * LROWS] = d.T.reshape(-1)
    tr_l = float(np.sum(diag / rowsum))
    tr_r = float(np.sum(diag / colsum))
    log_eps = math.log(EPS)
    log_1meps = math.log(1.0 - EPS)
    loss_l = -(tr_l * log_1meps + (B - tr_l) * log_eps)
    loss_r = -(tr_r * log_1meps + (B - tr_r) * log_eps)
    loss = WEIGHT * (loss_l + loss_r) / 2.0 / B
    return np.asarray(loss, dtype=np.float32)


def kernel(**inputs):
    res = _run(inputs, trace=False)
    return _combine(res.results)


def kernel_traced(**inputs):
    res = _run(inputs, trace=True)
    return _combine(res.results), res

